# revision 1
# baseline (speedup 1.0000x reference)
"""Trainium2 Bass kernel for nn_CNNNer (sparse band biaffine NER scorer).

Math collapse used here (everything after the GELU stage is linear):
  head = gelu(state@Wh+bh) ++ [1]          (features i = 0..200, i=200 is the 1)
  tail = gelu(state@Wt+bt) ++ [1]
  band[n,r,k] = head[n]^T U''_k tail[m],  m = n+r-64
      with U''_k = U_k + e_200 Wtp[k,:] + Whp[k,:]^T e_200^T
      (folds the h2/t2 additive terms of scores2 through the ones feature)
  scores'[n,r,t] = sum_k Wd[k,t] band_masked[n,r,k]
      masking zeroes whole head/tail feature columns (query/key validity),
      which commutes with the k-contraction, so
  scores'[n,r,t] = head_masked[n]^T UW_t tail_masked[m],
      UW_t = sum_k Wd[k,t] U''_k            (precomputed on host, [9,201,201])
  scores = scores' + bd  (host), masked-out entries = bd exactly.

Device work per core (8 cores; core = (batch b, query quarter) of 256 queries):
  1. headT/tailT = gelu MLPs computed transposed ([feature, position]).
  2. step A: UhT_t[j, x] = sum_i UW[t,i,j] headT[i,x]        (9 tags)
  3. step B: S_t[x, m]  = sum_j UhT_t[j, x] tailT[j, m]      (full 128x256
     score windows per query-chunk; band diag extracted on host)
"""

import os

import numpy as np

B, N, HID = 2, 1024, 768
BSZ = 200
W = 64
TAGS = 9
F = BSZ + 1  # 201 features incl the ones column
NQ = 256  # queries per core
NW = NQ + 2 * W  # 384 window positions per core
R = 2 * W + 1  # 129 band offsets
NCORES = 8
I2 = F - 128  # 73: second feature tile rows (i = 128..200)
F2 = BSZ - 128  # 72: second MLP output tile cols

_cache: dict = {}


def io_dt_name():
    return os.environ.get("BASSK_IO_DT", "f32r")


def _build_nc():
    import concourse.bass as bass
    import concourse.mybir as mybir
    import concourse.tile as tile
    from concourse import bacc

    dt = mybir.dt
    f32 = dt.float32
    io = {"f32": f32, "f32r": dt.float32r, "bf16": dt.bfloat16}[io_dt_name()]

    nc = bacc.Bacc(
        "TRN2", target_bir_lowering=False, debug=False, enable_asserts=False
    )
    xT = nc.dram_tensor("xT", [HID, NW], io, kind="ExternalInput").ap()
    wh = nc.dram_tensor("wh", [HID, BSZ], io, kind="ExternalInput").ap()
    wt = nc.dram_tensor("wt", [HID, BSZ], io, kind="ExternalInput").ap()
    # bias4 cols: bh[0:128], bt[0:128], bh[128:200]+pad, bt[128:200]+pad
    bias4 = nc.dram_tensor("bias4", [128, 4], f32, kind="ExternalInput").ap()
    # UW pre-arranged on host as [i, t, j] and split at i=128 so the loads
    # are plain row copies. j padded 201->204 so per-tag runs cannot merge
    # into descriptors over 1536B (those pin to a single DMA engine).
    FP = F + 3
    uw1d = nc.dram_tensor("uw1d", [128, TAGS, FP], io, kind="ExternalInput").ap()
    uw2d = nc.dram_tensor("uw2d", [I2, TAGS, FP], io, kind="ExternalInput").ap()
    # mask pre-broadcast on host: a partition-broadcast DMA lowers to
    # per-element descriptors and clogs the queue for ~25us
    msk = nc.dram_tensor("msk", [128, NW], io, kind="ExternalInput").ap()
    sout = nc.dram_tensor("sout", [TAGS, NQ, NQ], f32, kind="ExternalOutput").ap()

    gelu = {
        "gelu": mybir.ActivationFunctionType.Gelu,
        "identity": mybir.ActivationFunctionType.Identity,
    }[os.environ.get("BASSK_ACT", "gelu")]

    with tile.TileContext(nc) as tc:
        with (
            tc.tile_pool(name="sb", bufs=1) as sb,
            tc.tile_pool(name="ps_mlp", bufs=2, space="PSUM") as ps_mlp,
            tc.tile_pool(name="ps_a", bufs=2, space="PSUM") as ps_a,
            tc.tile_pool(name="ps_s", bufs=4, space="PSUM") as ps_s,
        ):
            # ---- loads (spread across DGE queues; x/weights split so the
            # MLP matmuls can start on the first chunks; uw queued behind
            # them so its transfer overlaps MLP compute) ----
            # One dma_start's descriptor chain runs on a single DMA engine
            # (~22.5 GB/s), so split each sizable load into pieces that run
            # on separate engines concurrently.
            qs = (nc.sync, nc.scalar)
            xTr = xT.rearrange("(ht p) c -> p ht c", p=128)
            x_sb = sb.tile([128, 6, NW], io)
            nc.sync.dma_start(out=x_sb[:, 0:3, :], in_=xTr[:, 0:3, :])
            nc.scalar.dma_start(out=x_sb[:, 3:6, :], in_=xTr[:, 3:6, :])
            whr = wh.rearrange("(ht p) m -> p ht m", p=128)
            wtr = wt.rearrange("(ht p) m -> p ht m", p=128)
            wh_sb = sb.tile([128, 6, BSZ], io)
            wt_sb = sb.tile([128, 6, BSZ], io)
            nc.sync.dma_start(out=wh_sb, in_=whr)
            nc.scalar.dma_start(out=wt_sb, in_=wtr)
            m_sb = sb.tile([128, NW], io)
            nc.gpsimd.dma_start(out=m_sb, in_=msk)
            b_sb = sb.tile([128, 4], f32)
            nc.gpsimd.dma_start(out=b_sb, in_=bias4)
            uw1 = sb.tile([128, TAGS, F], io)
            uw2 = sb.tile([I2, TAGS, F], io)
            nc.sync.dma_start(out=uw1, in_=uw1d[:, :, 0:F])
            nc.scalar.dma_start(out=uw2, in_=uw2d[:, :, 0:F])
            bh1, bt1 = b_sb[:, 0:1], b_sb[:, 1:2]
            bh2, bt2 = b_sb[0:F2, 2:3], b_sb[0:F2, 3:4]

            headT1 = sb.tile([128, NQ], io)
            headT2 = sb.tile([I2, NQ], io)
            tailT1 = sb.tile([128, NW], io)
            tailT2 = sb.tile([I2, NW], io)
            uh1 = sb.tile([128, TAGS, NQ], io)
            uh2 = sb.tile([I2, TAGS, NQ], io)
            s_sb0 = sb.tile([128, TAGS, NQ], f32)
            s_sb1 = sb.tile([128, TAGS, NQ], f32)

            # ---- MLPs: o = gelu(W^T x + b), computed transposed ----
            for w_sb, b1, b2, o1, o2, c0, ncols in (
                (wh_sb, bh1, bh2, headT1, headT2, W, NQ),
                (wt_sb, bt1, bt2, tailT1, tailT2, 0, NW),
            ):
                for fw, f0, o, bias in ((128, 0, o1, b1), (F2, 128, o2, b2)):
                    pm = ps_mlp.tile([fw, ncols], f32, tag="pm")
                    for ht in range(6):
                        nc.tensor.matmul(
                            pm,
                            w_sb[:, ht, f0 : f0 + fw],
                            x_sb[:, ht, c0 : c0 + ncols],
                            start=(ht == 0),
                            stop=(ht == 5),
                        )
                    nc.scalar.activation(out=o[0:fw, :], in_=pm, func=gelu, bias=bias)
                # mask all columns; ones feature row (i == 200) is the mask
                # row itself, DMA'd in (engines can't address partition 72)
                nc.vector.tensor_mul(o1, o1, m_sb[0:128, c0 : c0 + ncols])
                nc.vector.tensor_mul(
                    o2[0:F2, :], o2[0:F2, :], m_sb[0:F2, c0 : c0 + ncols]
                )
                nc.gpsimd.dma_start(
                    out=o2[F2 : F2 + 1, :], in_=msk[0:1, c0 : c0 + ncols]
                )

            # ---- step A: UhT_t[j, x] = sum_i UW[t,i,j] headT[i,x] ----
            for t in range(TAGS):
                for jw, j0, uh in ((128, 0, uh1), (I2, 128, uh2)):
                    pa = ps_a.tile([jw, NQ], f32, tag="pa")
                    for it, (u_sb, h_sb) in enumerate(
                        ((uw1, headT1), (uw2, headT2))
                    ):
                        nc.tensor.matmul(
                            pa,
                            u_sb[:, t, j0 : j0 + jw],
                            h_sb,
                            start=(it == 0),
                            stop=(it == 1),
                        )
                    nc.any.tensor_copy(uh[:, t, :], pa)

            # ---- step B: S_t[x, m] = sum_j UhT_t[j, x] tailT[j, m] ----
            for qc in range(2):
                s_sb = (s_sb0, s_sb1)[qc]
                for t in range(TAGS):
                    pS = ps_s.tile([128, NQ], f32, tag="ps")
                    for jt, (uh, tl) in enumerate(((uh1, tailT1), (uh2, tailT2))):
                        nc.tensor.matmul(
                            pS,
                            uh[:, t, qc * 128 : qc * 128 + 128],
                            tl[:, qc * 128 : qc * 128 + NQ],
                            start=(jt == 0),
                            stop=(jt == 1),
                        )
                    nc.any.tensor_copy(s_sb[:, t, :], pS)
                    if t % 3 == 2:
                        # store finished tag-triples so writeback overlaps
                        # the remaining compute
                        qs[(qc + t) % 2].dma_start(
                            out=sout[
                                t - 2 : t + 1, qc * 128 : (qc + 1) * 128, :
                            ].transpose([1, 0, 2]),
                            in_=s_sb[:, t - 2 : t + 1, :],
                        )

    nc.compile()
    return nc


def _np_io_dt():
    if io_dt_name() == "bf16":
        import ml_dtypes

        return ml_dtypes.bfloat16
    return np.float32


def _get_nc():
    key = "nc-" + io_dt_name()
    if key not in _cache:
        _cache[key] = _build_nc()
    return _cache[key]


def _install_ntff_hook():
    """Profiling-only (BASSK_TRACE=1): provide antenv.axon_hooks if the
    image lacks it, wired to the libaxon NTFF capture via ctypes."""
    import sys
    import types

    try:
        from antenv.axon_hooks import get_axon_ntff_profile_hook  # noqa: F401

        return
    except ImportError:
        pass
    from trn_agent_boot.trn_boot import _ntff_profile_via_ctypes

    hook = _ntff_profile_via_ctypes("/opt/axon/libaxon_pjrt.so")
    mod = types.ModuleType("antenv.axon_hooks")
    mod._hook = hook
    mod.get_axon_ntff_profile_hook = lambda: mod._hook
    mod.set_axon_ntff_profile_hook = lambda h: setattr(mod, "_hook", h)
    sys.modules["antenv.axon_hooks"] = mod


def _host_prep(state, lengths, Wh, bh, Wt, bt, U, Wcat, Wd):
    """Fold U/Wcat/Wd into UW[9,201,201] and build per-core inputs."""
    Whp = Wcat[:, :F]  # [K, 201]
    Wtp = Wcat[:, F:]  # [K, 201]
    U2 = U.astype(np.float64).copy()
    U2[:, F - 1, :] += Wtp  # head ones-row picks up the tail term
    U2[:, :, F - 1] += Whp  # tail ones-col picks up the head term
    UW = np.einsum("kt,kij->tij", Wd.astype(np.float64), U2).astype(np.float32)
    UW = np.ascontiguousarray(UW)

    in_maps = []
    for b in range(B):
        for qi in range(N // NQ):
            q0 = qi * NQ
            lo = q0 - W
            xw = np.zeros((NW, HID), np.float32)
            s, e = max(lo, 0), min(q0 + NQ + W, N)
            xw[s - lo : e - lo] = state[b, s:e]
            pos = lo + np.arange(NW)
            mrow = ((pos >= 0) & (pos < N) & (pos < lengths[b])).astype(np.float32)
            iodt = _np_io_dt()
            uwp = np.zeros((F, TAGS, F + 3), UW.dtype)
            uwp[:, :, 0:F] = UW.transpose(1, 0, 2)
            uwr = uwp.astype(iodt)
            in_maps.append(
                {
                    "xT": np.ascontiguousarray(xw.T).astype(iodt),
                    "wh": Wh.astype(iodt),
                    "wt": Wt.astype(iodt),
                    "bias4": np.ascontiguousarray(
                        np.stack(
                            [
                                bh[0:128],
                                bt[0:128],
                                np.pad(bh[128:BSZ], (0, 128 - F2)),
                                np.pad(bt[128:BSZ], (0, 128 - F2)),
                            ],
                            axis=1,
                        ).astype(np.float32)
                    ),
                    "uw1d": np.ascontiguousarray(uwr[0:128]),
                    "uw2d": np.ascontiguousarray(uwr[128:F]),
                    "msk": np.ascontiguousarray(
                        np.broadcast_to(mrow[None, :], (128, NW))
                    ).astype(iodt),
                }
            )
    return in_maps


def _assemble(outs, bd):
    """outs: NCORES arrays [TAGS, NQ, NQ] -> scores [B, N, R, TAGS]."""
    scores = np.empty((B, N, R, TAGS), np.float32)
    mi = (np.arange(NQ) % 128)[:, None] + np.arange(R)[None, :]
    for c, S in enumerate(outs):
        b, qi = divmod(c, N // NQ)
        g = np.take_along_axis(S, mi[None, :, :], axis=2)
        scores[b, qi * NQ : (qi + 1) * NQ] = g.transpose(1, 2, 0)
    scores += bd.astype(np.float32)[None, None, None, :]
    return np.where(np.isfinite(scores), scores, 0.0).astype(np.float32)


def kernel(**inputs):
    state = np.asarray(inputs["state"], np.float32)
    lengths = np.asarray(inputs["lengths"]).astype(np.int64)
    Wh = np.ascontiguousarray(np.asarray(inputs["Wh"], np.float32))
    bh = np.asarray(inputs["bh"], np.float32)
    Wt = np.ascontiguousarray(np.asarray(inputs["Wt"], np.float32))
    bt = np.asarray(inputs["bt"], np.float32)
    U = np.asarray(inputs["U"], np.float32)
    Wcat = np.asarray(inputs["Wcat"], np.float32)
    Wd = np.asarray(inputs["Wd"], np.float32)
    bd = np.asarray(inputs["bd"], np.float32)

    in_maps = _host_prep(state, lengths, Wh, bh, Wt, bt, U, Wcat, Wd)
    nc = _get_nc()

    if os.environ.get("BASSK_SIM"):
        from concourse.bass_interp import CoreSim

        outs = []
        for im in in_maps:
            sim = CoreSim(nc, trace=False)
            for k, v in im.items():
                sim.tensor(k)[:] = v
            sim.simulate()
            outs.append(sim.tensor("sout").copy())
    else:
        trace = bool(os.environ.get("BASSK_TRACE"))
        if trace:
            _install_ntff_hook()
        from concourse.bass_utils import run_bass_kernel_spmd

        try:
            res = run_bass_kernel_spmd(
                nc, in_maps, core_ids=list(range(NCORES)), trace=trace
            )
        except Exception:
            # transient NRT/device hiccups recover on a fresh attempt
            import time

            time.sleep(2.0)
            res = run_bass_kernel_spmd(
                nc, in_maps, core_ids=list(range(NCORES)), trace=trace
            )
        _cache["last_result"] = res
        outs = [r["sout"] for r in res.results]

    return _assemble(outs, bd)



# revision 14
# speedup vs baseline: 1.1862x; 1.1862x over previous
"""Trainium2 Bass kernel for nn_CNNNer (sparse band biaffine NER scorer).

Math collapse (everything after the GELU stage is linear):
  head = gelu(state@Wh+bh) ++ [1]          (features i = 0..200, i=200 is the 1)
  tail = gelu(state@Wt+bt) ++ [1]
  band[n,r,k] = head[n]^T U''_k tail[m],  m = n+r-64
      with U''_k = U_k + e_200 Wtp[k,:] + Whp[k,:]^T e_200^T
  scores'[n,r,t] = head[n]^T UW_t tail[m],  UW_t = sum_k Wd[k,t] U''_k
      (precomputed on host, [9,201,201]); scores = scores' + bd.
  Pad masking only ever zeroes whole band entries -> masked scores equal bd
  exactly, so masking moves to the host entirely (device computes garbage in
  masked slots; finite, overwritten on host).

Device work per core (8 cores; core = (batch b, query quarter), 256 queries,
window of NW=384 key positions), all matmuls bf16 with fp32 PSUM accumulate:
  MLP:   headT[f, x] (queries, 256 cols), tailT[f, m] (384 cols), f = 201
         (128 + 73 partition tiles; row 200 memset to 1.0).
  A:     uhT_t[j, x] = sum_i UW[t,i,j] headT[i,x]   (9 tags, 36 matmuls)
  B:     S_t[m, x]   = sum_j tailT[j, m] uhT_t[j, x], computed as 4 groups
         (m-block 0 x-lo, m-block 1 x-lo, m-block 1 x-hi, m-block 2 x-hi),
         tail slice stationary, uh (t,x)-chunks of 512 moving.
Output sout[4, 128, 1152] bf16 = [group][m][(t,x)]; host extracts the
129-wide band diagonals, applies pad mask and + bd.
"""

import os

import numpy as np

B, N, HID = 2, 1024, 768
BSZ = 200
W = 64
TAGS = 9
F = BSZ + 1  # 201 features incl the ones column
NQ = 256  # queries per core
NW = NQ + 2 * W  # 384 window positions per core
R = 2 * W + 1  # 129 band offsets
NCORES = 8
I2 = F - 128  # 73: second feature tile rows (incl ones row at local 72)
F2 = BSZ - 128  # 72: second MLP output tile rows
GSZ = TAGS * 128  # 1152: per-group output elems per partition

_cache: dict = {}


def io_dt_name():
    return os.environ.get("BASSK_IO_DT", "bf16")


def _build_nc():
    import concourse.bass as bass
    import concourse.mybir as mybir
    import concourse.tile as tile
    from concourse import bacc

    dt = mybir.dt
    f32 = dt.float32
    io = {"f32": f32, "f32r": dt.float32r, "bf16": dt.bfloat16}[io_dt_name()]
    nwarm = int(os.environ.get("BASSK_WARM", "6"))

    nc = bacc.Bacc(
        "TRN2", target_bir_lowering=False, debug=False, enable_asserts=False
    )
    # All dram layouts are partition-major with per-partition contiguous runs
    # of 2.3-4.6KB so DMA descriptors are fat and stripe across the 16 DMA
    # engines.
    xd = nc.dram_tensor("xd", [128, 6, NW], io, kind="ExternalInput").ap()
    whd = nc.dram_tensor("whd", [128, 6, BSZ], io, kind="ExternalInput").ap()
    wtd = nc.dram_tensor("wtd", [128, 6, BSZ], io, kind="ExternalInput").ap()
    # bias4 cols: bh[0:128], bt[0:128], bh[128:200]+pad, bt[128:200]+pad
    bias4 = nc.dram_tensor("bias4", [128, 4], f32, kind="ExternalInput").ap()
    uw1d = nc.dram_tensor("uw1d", [128, TAGS, F], io, kind="ExternalInput").ap()
    uw2d = nc.dram_tensor("uw2d", [I2, TAGS, F], io, kind="ExternalInput").ap()
    onesd = nc.dram_tensor("onesd", [1, NW], io, kind="ExternalInput").ap()
    sout = nc.dram_tensor("sout", [4, 128, GSZ], io, kind="ExternalOutput").ap()

    gelu = {
        "gelu": mybir.ActivationFunctionType.Gelu,
        "identity": mybir.ActivationFunctionType.Identity,
    }[os.environ.get("BASSK_ACT", "gelu")]
    copyf = mybir.ActivationFunctionType.Copy

    with tile.TileContext(nc) as tc:
        with (
            tc.tile_pool(name="sb", bufs=1) as sb,
            tc.tile_pool(name="ps", bufs=1, space="PSUM") as ps,
        ):
            x_sb = sb.tile([128, 6, NW], io)
            wh_sb = sb.tile([128, 6, BSZ], io)
            wt_sb = sb.tile([128, 6, BSZ], io)
            b_sb = sb.tile([128, 4], f32)
            uw1 = sb.tile([128, TAGS, F], io)
            uw2 = sb.tile([I2, TAGS, F], io)
            headT1 = sb.tile([128, NQ], io)
            headT2 = sb.tile([I2, NQ], io)
            tailT1 = sb.tile([128, NW], io)
            tailT2 = sb.tile([I2, NW], io)
            uh1 = sb.tile([128, TAGS, NQ], io)
            uh2 = sb.tile([I2, TAGS, NQ], io)
            s_sb = sb.tile([128, 4, GSZ], io)
            warm = sb.tile([128, 512], io)

            # ---- loads: uw stream on gpsimd queue, the rest on sync queue
            # in first-use order so the MLP can start on early pieces ----
            nc.gpsimd.dma_start(out=b_sb, in_=bias4)
            nc.gpsimd.dma_start(out=uw1, in_=uw1d)
            nc.gpsimd.dma_start(out=uw2, in_=uw2d)
            nc.sync.dma_start(out=wh_sb[:, 0:3, :], in_=whd[:, 0:3, :])
            nc.sync.dma_start(out=x_sb[:, 0:3, :], in_=xd[:, 0:3, :])
            nc.sync.dma_start(out=wh_sb[:, 3:6, :], in_=whd[:, 3:6, :])
            nc.sync.dma_start(out=x_sb[:, 3:6, :], in_=xd[:, 3:6, :])
            nc.sync.dma_start(out=wt_sb[:, 0:3, :], in_=wtd[:, 0:3, :])
            nc.sync.dma_start(out=wt_sb[:, 3:6, :], in_=wtd[:, 3:6, :])

            # ones feature rows (feature 200 = local row 72 of tile 2);
            # engines can't address partition start 72, so DMA them in
            nc.gpsimd.dma_start(out=headT2[F2:I2, :], in_=onesd[:, 0:NQ])
            nc.gpsimd.dma_start(out=tailT2[F2:I2, :], in_=onesd)
            nc.vector.memset(warm, 0.5)

            # All PSUM comes from one tag of [128, 1024]-f32 (2-bank) slots,
            # 4 bufs = all 8 banks. Each accumulation group starts at a bank
            # boundary (a group's start-flag zeroes its whole 2KB bank).
            def pslot():
                return ps.tile([128, 2, 512], f32, tag="u", bufs=4, name="pu")

            # ---- PE clock warmup: dependency-free matmuls run while the
            # input DMAs stream, ramping the tensor engine out of its low
            # p-state before the real work arrives ----
            if nwarm:
                pw = pslot()
                for k in range(nwarm):
                    nc.tensor.matmul(
                        pw[:, 0, :], warm[:, 0:128], warm,
                        start=(k == 0), stop=(k == nwarm - 1),
                    )

            # ---- MLPs: o = gelu(W^T x + b), transposed layout; head (12
            # matmuls) first so step A can start while the tail runs ----
            pm_h = pslot()
            pm_t = pslot()
            for ht in range(6):
                fl = (ht == 0, ht == 5)
                nc.tensor.matmul(
                    pm_h[:, 0, 0:NQ], wh_sb[:, ht, 0:128],
                    x_sb[:, ht, W : W + NQ], start=fl[0], stop=fl[1],
                )
                nc.tensor.matmul(
                    pm_h[0:F2, 1, 0:NQ], wh_sb[:, ht, 128:BSZ],
                    x_sb[:, ht, W : W + NQ], start=fl[0], stop=fl[1],
                )
            nc.scalar.activation(
                out=headT1, in_=pm_h[:, 0, 0:NQ], func=gelu, bias=b_sb[:, 0:1]
            )
            nc.scalar.activation(
                out=headT2[0:F2, :], in_=pm_h[0:F2, 1, 0:NQ], func=gelu,
                bias=b_sb[0:F2, 2:3],
            )
            for ht in range(6):
                fl = (ht == 0, ht == 5)
                nc.tensor.matmul(
                    pm_t[:, 0, 0:NW], wt_sb[:, ht, 0:128], x_sb[:, ht, :],
                    start=fl[0], stop=fl[1],
                )
                nc.tensor.matmul(
                    pm_t[0:F2, 1, 0:NW], wt_sb[:, ht, 128:BSZ], x_sb[:, ht, :],
                    start=fl[0], stop=fl[1],
                )
            nc.scalar.activation(
                out=tailT1, in_=pm_t[:, 0, 0:NW], func=gelu, bias=b_sb[:, 1:2]
            )
            nc.scalar.activation(
                out=tailT2[0:F2, :], in_=pm_t[0:F2, 1, 0:NW], func=gelu,
                bias=b_sb[0:F2, 3:4],
            )

            # ---- step A: uhT_t[j, x] = sum_i UW[t,i,j] headT[i,x]; tag
            # pairs share one PSUM bank so copies are 512 wide ----
            # only vector (DVE) and scalar (ACT) can read PSUM
            def pcopy(k, out, in_):
                if k % 2:
                    nc.scalar.activation(out=out, in_=in_, func=copyf)
                else:
                    nc.vector.tensor_copy(out, in_)

            for jt, (jw, j0, uh) in enumerate(((128, 0, uh1), (I2, 128, uh2))):
                for tp in range(5):
                    t0, tn = 2 * tp, min(2, TAGS - 2 * tp)
                    pa = pslot()
                    for tt in range(tn):
                        for it, (u_sb, h_sb) in enumerate(
                            ((uw1, headT1), (uw2, headT2))
                        ):
                            nc.tensor.matmul(
                                pa[0:jw, tt, 0:NQ],
                                u_sb[:, t0 + tt, j0 : j0 + jw],
                                h_sb,
                                start=(it == 0), stop=(it == 1),
                            )
                    pcopy(
                        jt * 5 + tp,
                        uh[0:jw, t0 : t0 + tn, :], pa[0:jw, 0:tn, 0:NQ]
                    )

            # ---- step B: S[m, (t,x)] = sum_j tailT[j, m] uhT[j, (t,x)];
            # tail slice stationary, uh chunks of 512 moving, 4 groups ----
            # groups: (m-block, x-half): (0, lo), (1, lo), (1, hi), (2, hi)
            groups = ((0, 0), (1, 0), (1, 1), (2, 1))
            # chunks of the 9*128 free dim: (slot, bank, tag0, ntags)
            chunks = ((0, 0, 0, 4), (0, 1, 4, 4), (1, 0, 8, 1))
            sq = (nc.sync, nc.gpsimd, nc.sync, nc.gpsimd)
            for g, (mb, xh) in enumerate(groups):
                m0 = mb * 128
                x0 = xh * 128
                pb = (pslot(), pslot())
                for jt, (jw, tl, uh) in enumerate(
                    ((128, tailT1, uh1), (I2, tailT2, uh2))
                ):
                    for sl, bk, ct, cn in chunks:
                        nc.tensor.matmul(
                            pb[sl][:, bk, 0 : cn * 128],
                            tl[0:jw, m0 : m0 + 128],
                            uh[0:jw, ct : ct + cn, x0 : x0 + 128],
                            start=(jt == 0), stop=(jt == 1),
                        )
                pcopy(g, s_sb[:, g, 0:1024], pb[0].rearrange("p a b -> p (a b)"))
                pcopy(g + 1, s_sb[:, g, 1024:GSZ], pb[1][:, 0, 0:128])
                sq[g].dma_start(out=sout[g], in_=s_sb[:, g, :])

    nc.compile()
    return nc


def _np_io_dt():
    if io_dt_name() == "bf16":
        import ml_dtypes

        return ml_dtypes.bfloat16
    return np.float32


def _get_nc():
    key = "nc-" + io_dt_name() + os.environ.get("BASSK_WARM", "6")
    if key not in _cache:
        _cache[key] = _build_nc()
    return _cache[key]


def _install_ntff_hook():
    """Profiling-only (BASSK_TRACE=1): provide antenv.axon_hooks if the
    image lacks it, wired to the libaxon NTFF capture via ctypes."""
    import sys
    import types

    try:
        from antenv.axon_hooks import get_axon_ntff_profile_hook  # noqa: F401

        return
    except ImportError:
        pass
    from trn_agent_boot.trn_boot import _ntff_profile_via_ctypes

    hook = _ntff_profile_via_ctypes("/opt/axon/libaxon_pjrt.so")
    mod = types.ModuleType("antenv.axon_hooks")
    mod._hook = hook
    mod.get_axon_ntff_profile_hook = lambda: mod._hook
    mod.set_axon_ntff_profile_hook = lambda h: setattr(mod, "_hook", h)
    sys.modules["antenv.axon_hooks"] = mod


def _host_prep(state, Wh, bh, Wt, bt, U, Wcat, Wd):
    """Fold U/Wcat/Wd into UW[9,201,201] and build per-core inputs."""
    iodt = _np_io_dt()
    Whp = Wcat[:, :F]  # [K, 201]
    Wtp = Wcat[:, F:]  # [K, 201]
    U2 = U.astype(np.float64).copy()
    U2[:, F - 1, :] += Wtp  # head ones-row picks up the tail term
    U2[:, :, F - 1] += Whp  # tail ones-col picks up the head term
    UW = np.einsum("kt,kij->tij", Wd.astype(np.float64), U2).astype(np.float32)
    UWt = np.ascontiguousarray(UW.transpose(1, 0, 2))  # [i, t, j]

    whd = np.ascontiguousarray(
        Wh.reshape(6, 128, BSZ).transpose(1, 0, 2)
    ).astype(iodt)
    wtd = np.ascontiguousarray(
        Wt.reshape(6, 128, BSZ).transpose(1, 0, 2)
    ).astype(iodt)
    bias4 = np.ascontiguousarray(
        np.stack(
            [
                bh[0:128],
                bt[0:128],
                np.pad(bh[128:BSZ], (0, 128 - F2)),
                np.pad(bt[128:BSZ], (0, 128 - F2)),
            ],
            axis=1,
        ).astype(np.float32)
    )
    uw1d = np.ascontiguousarray(UWt[0:128]).astype(iodt)
    uw2d = np.ascontiguousarray(UWt[128:F]).astype(iodt)
    onesd = np.ones((1, NW), iodt)

    in_maps = []
    for b in range(B):
        for qi in range(N // NQ):
            lo = qi * NQ - W
            xw = np.zeros((NW, HID), np.float32)
            s, e = max(lo, 0), min(lo + NW, N)
            xw[s - lo : e - lo] = state[b, s:e]
            xdev = np.ascontiguousarray(
                xw.T.reshape(6, 128, NW).transpose(1, 0, 2)
            ).astype(iodt)
            in_maps.append(
                {
                    "xd": xdev,
                    "whd": whd,
                    "wtd": wtd,
                    "bias4": bias4,
                    "uw1d": uw1d,
                    "uw2d": uw2d,
                    "onesd": onesd,
                }
            )
    return in_maps


def _assemble(outs, bd, lengths):
    """outs: NCORES arrays [4, 128, TAGS*128] -> scores [B, N, R, TAGS]."""
    n_ar = np.arange(N)
    offs = np.arange(R) - W
    j_idx = n_ar[:, None] + offs[None, :]  # [N, R]
    in_range = (j_idx >= 0) & (j_idx < N)
    key_ok = in_range[None] & (j_idx[None] < lengths[:, None, None])
    q_ok = n_ar[None, :] < lengths[:, None]
    pad = ~(key_ok & q_ok[:, :, None])  # [B, N, R]

    xx = np.arange(128)
    idx = (xx[:, None] + np.arange(R)[None, :])[:, :, None]  # [128, R, 1]
    scores = np.empty((B, N, R, TAGS), np.float32)
    for c, S in enumerate(outs):
        b, qi = divmod(c, N // NQ)
        q0 = qi * NQ
        G = S.astype(np.float32).reshape(4, 128, TAGS, 128)
        for half in range(2):
            H = np.concatenate(
                (G[2 * half], G[2 * half + 1]), axis=0
            )  # [256 m, TAGS, 128 xx]
            T = H.transpose(2, 0, 1)  # [xx, m, t]
            band = np.take_along_axis(T, idx, axis=1)  # [128, R, TAGS]
            scores[b, q0 + 128 * half : q0 + 128 * (half + 1)] = band
    scores = np.where(pad[..., None], 0.0, scores) + bd.astype(np.float32)
    return np.where(np.isfinite(scores), scores, 0.0).astype(np.float32)


def kernel(**inputs):
    state = np.asarray(inputs["state"], np.float32)
    lengths = np.asarray(inputs["lengths"]).astype(np.int64)
    Wh = np.ascontiguousarray(np.asarray(inputs["Wh"], np.float32))
    bh = np.asarray(inputs["bh"], np.float32)
    Wt = np.ascontiguousarray(np.asarray(inputs["Wt"], np.float32))
    bt = np.asarray(inputs["bt"], np.float32)
    U = np.asarray(inputs["U"], np.float32)
    Wcat = np.asarray(inputs["Wcat"], np.float32)
    Wd = np.asarray(inputs["Wd"], np.float32)
    bd = np.asarray(inputs["bd"], np.float32)

    in_maps = _host_prep(state, Wh, bh, Wt, bt, U, Wcat, Wd)
    nc = _get_nc()

    if os.environ.get("BASSK_SIM"):
        from concourse.bass_interp import CoreSim

        outs = []
        for im in in_maps:
            sim = CoreSim(nc, trace=False)
            for k, v in im.items():
                sim.tensor(k)[:] = v
            sim.simulate()
            outs.append(sim.tensor("sout").copy())
    else:
        trace = bool(os.environ.get("BASSK_TRACE"))
        if trace:
            _install_ntff_hook()
        from concourse.bass_utils import run_bass_kernel_spmd

        try:
            res = run_bass_kernel_spmd(
                nc, in_maps, core_ids=list(range(NCORES)), trace=trace
            )
        except Exception:
            # transient NRT/device hiccups recover on a fresh attempt
            import time

            time.sleep(2.0)
            res = run_bass_kernel_spmd(
                nc, in_maps, core_ids=list(range(NCORES)), trace=trace
            )
        _cache["last_result"] = res
        outs = [r["sout"] for r in res.results]

    return _assemble(outs, bd, lengths)


# revision 17
# speedup vs baseline: 1.2357x; 1.0417x over previous
"""Trainium2 Bass kernel for nn_CNNNer (sparse band biaffine NER scorer).

Math collapse (everything after the GELU stage is linear):
  head = gelu(state@Wh+bh) ++ [1]          (features i = 0..200, i=200 is the 1)
  tail = gelu(state@Wt+bt) ++ [1]
  band[n,r,k] = head[n]^T U''_k tail[m],  m = n+r-64
      with U''_k = U_k + e_200 Wtp[k,:] + Whp[k,:]^T e_200^T
  scores'[n,r,t] = head[n]^T UW_t tail[m],  UW_t = sum_k Wd[k,t] U''_k
      (precomputed on host, [9,201,201]); scores = scores' + bd.
  Pad masking only ever zeroes whole band entries -> masked scores equal bd
  exactly, so masking moves to the host entirely (device computes garbage in
  masked slots; finite, overwritten on host).

Device work per core (8 cores; core = (batch b, query quarter), 256 queries,
window of NW=384 key positions), all matmuls bf16 with fp32 PSUM accumulate:
  MLP:   headT[f, x] (queries, 256 cols), tailT[f, m] (384 cols), f = 201
         (128 + 73 partition tiles; row 200 memset to 1.0).
  A:     uhT_t[j, x] = sum_i UW[t,i,j] headT[i,x]   (9 tags, 36 matmuls)
  B:     S_t[m, x]   = sum_j tailT[j, m] uhT_t[j, x], computed as 4 groups
         (m-block 0 x-lo, m-block 1 x-lo, m-block 1 x-hi, m-block 2 x-hi),
         tail slice stationary, uh (t,x)-chunks of 512 moving.
Output sout[4, 128, 1152] bf16 = [group][m][(t,x)]; host extracts the
129-wide band diagonals, applies pad mask and + bd.
"""

import os

import numpy as np

B, N, HID = 2, 1024, 768
BSZ = 200
W = 64
TAGS = 9
F = BSZ + 1  # 201 features incl the ones column
NQ = 256  # queries per core
NW = NQ + 2 * W  # 384 window positions per core
R = 2 * W + 1  # 129 band offsets
NCORES = 8
I2 = F - 128  # 73: second feature tile rows (incl ones row at local 72)
F2 = BSZ - 128  # 72: second MLP output tile rows
GSZ = TAGS * 128  # 1152: per-group output elems per partition

_cache: dict = {}


def io_dt_name():
    return os.environ.get("BASSK_IO_DT", "bf16")


def _build_nc():
    import concourse.bass as bass
    import concourse.mybir as mybir
    import concourse.tile as tile
    from concourse import bacc

    dt = mybir.dt
    f32 = dt.float32
    io = {"f32": f32, "f32r": dt.float32r, "bf16": dt.bfloat16}[io_dt_name()]
    nwarm = int(os.environ.get("BASSK_WARM", "6"))

    nc = bacc.Bacc(
        "TRN2", target_bir_lowering=False, debug=False, enable_asserts=False
    )
    # All dram layouts are partition-major with per-partition contiguous runs
    # of 2.3-4.6KB so DMA descriptors are fat and stripe across the 16 DMA
    # engines.
    xd = nc.dram_tensor("xd", [128, 6, NW], io, kind="ExternalInput").ap()
    whd = nc.dram_tensor("whd", [128, 6, BSZ], io, kind="ExternalInput").ap()
    wtd = nc.dram_tensor("wtd", [128, 6, BSZ], io, kind="ExternalInput").ap()
    # bias4 cols: bh[0:128], bt[0:128], bh[128:200]+pad, bt[128:200]+pad
    bias4 = nc.dram_tensor("bias4", [128, 4], f32, kind="ExternalInput").ap()
    # uw rows are 1809 elems; stored as 3 padded runs of 603 so DMA packets
    # stay under 1536B (larger ones skew onto a single DMA engine)
    uw1d = nc.dram_tensor("uw1d", [128, 3, 608], io, kind="ExternalInput").ap()
    uw2d = nc.dram_tensor("uw2d", [I2, 3, 608], io, kind="ExternalInput").ap()
    onesd = nc.dram_tensor("onesd", [1, NW], io, kind="ExternalInput").ap()
    sout = nc.dram_tensor("sout", [4, 128, GSZ], io, kind="ExternalOutput").ap()

    gelu = {
        "gelu": mybir.ActivationFunctionType.Gelu,
        "identity": mybir.ActivationFunctionType.Identity,
    }[os.environ.get("BASSK_ACT", "gelu")]
    copyf = mybir.ActivationFunctionType.Copy

    with tile.TileContext(nc) as tc:
        with (
            tc.tile_pool(name="sb", bufs=1) as sb,
            tc.tile_pool(name="ps", bufs=1, space="PSUM") as ps,
        ):
            x_sb = sb.tile([128, 6, NW], io)
            wh_sb = sb.tile([128, 6, BSZ], io)
            wt_sb = sb.tile([128, 6, BSZ], io)
            b_sb = sb.tile([128, 4], f32)
            uw1 = sb.tile([128, TAGS, F], io)
            uw2 = sb.tile([I2, TAGS, F], io)
            headT1 = sb.tile([128, NQ], io)
            headT2 = sb.tile([I2, NQ], io)
            tailT1 = sb.tile([128, NW], io)
            tailT2 = sb.tile([I2, NW], io)
            uh1 = sb.tile([128, TAGS, NQ], io)
            uh2 = sb.tile([I2, TAGS, NQ], io)
            s_sb = sb.tile([128, 4, GSZ], io)
            warm = sb.tile([128, 512], io)

            # ---- loads: x/wh/wt stream on the sync queue in first-use
            # order so the MLP can start early; uw thirds split between
            # gpsimd and scalar queues so they stripe and land before A ----
            uw1f = uw1.rearrange("p t f -> p (t f)")
            uw2f = uw2.rearrange("p t f -> p (t f)")
            nc.gpsimd.dma_start(out=headT2[F2:I2, :], in_=onesd[:, 0:NQ])
            nc.gpsimd.dma_start(out=tailT2[F2:I2, :], in_=onesd)
            nc.gpsimd.dma_start(out=b_sb, in_=bias4)
            for k in range(3):
                nc.gpsimd.dma_start(
                    out=uw1f[:, 603 * k : 603 * (k + 1)],
                    in_=uw1d[:, k, 0:603],
                )
                nc.scalar.dma_start(
                    out=uw2f[0:I2, 603 * k : 603 * (k + 1)],
                    in_=uw2d[:, k, 0:603],
                )
            nc.sync.dma_start(out=wh_sb[:, 0:3, :], in_=whd[:, 0:3, :])
            nc.sync.dma_start(out=x_sb[:, 0:3, :], in_=xd[:, 0:3, :])
            nc.sync.dma_start(out=wh_sb[:, 3:6, :], in_=whd[:, 3:6, :])
            nc.sync.dma_start(out=x_sb[:, 3:6, :], in_=xd[:, 3:6, :])
            nc.sync.dma_start(out=wt_sb[:, 0:3, :], in_=wtd[:, 0:3, :])
            nc.sync.dma_start(out=wt_sb[:, 3:6, :], in_=wtd[:, 3:6, :])

            nc.vector.memset(warm, 0.5)

            # All PSUM comes from one tag of [128, 1024]-f32 (2-bank) slots,
            # 4 bufs = all 8 banks. Each accumulation group starts at a bank
            # boundary (a group's start-flag zeroes its whole 2KB bank).
            def pslot():
                return ps.tile([128, 2, 512], f32, tag="u", bufs=4, name="pu")

            # ---- PE clock warmup: dependency-free matmuls run while the
            # input DMAs stream, ramping the tensor engine out of its low
            # p-state before the real work arrives ----
            if nwarm:
                pw = pslot()
                for k in range(nwarm):
                    nc.tensor.matmul(
                        pw[:, 0, :], warm[:, 0:128], warm,
                        start=(k == 0), stop=(k == nwarm - 1),
                    )

            # ---- MLPs: o = gelu(W^T x + b), transposed layout; head (12
            # matmuls) first so step A can start while the tail runs ----
            pm_h = pslot()
            pm_t = pslot()
            for ht in range(6):
                fl = (ht == 0, ht == 5)
                nc.tensor.matmul(
                    pm_h[:, 0, 0:NQ], wh_sb[:, ht, 0:128],
                    x_sb[:, ht, W : W + NQ], start=fl[0], stop=fl[1],
                )
                nc.tensor.matmul(
                    pm_h[0:F2, 1, 0:NQ], wh_sb[:, ht, 128:BSZ],
                    x_sb[:, ht, W : W + NQ], start=fl[0], stop=fl[1],
                )
            nc.scalar.activation(
                out=headT1, in_=pm_h[:, 0, 0:NQ], func=gelu, bias=b_sb[:, 0:1]
            )
            nc.scalar.activation(
                out=headT2[0:F2, :], in_=pm_h[0:F2, 1, 0:NQ], func=gelu,
                bias=b_sb[0:F2, 2:3],
            )
            for ht in range(6):
                fl = (ht == 0, ht == 5)
                nc.tensor.matmul(
                    pm_t[:, 0, 0:NW], wt_sb[:, ht, 0:128], x_sb[:, ht, :],
                    start=fl[0], stop=fl[1],
                )
                nc.tensor.matmul(
                    pm_t[0:F2, 1, 0:NW], wt_sb[:, ht, 128:BSZ], x_sb[:, ht, :],
                    start=fl[0], stop=fl[1],
                )
            nc.scalar.activation(
                out=tailT1, in_=pm_t[:, 0, 0:NW], func=gelu, bias=b_sb[:, 1:2]
            )
            nc.scalar.activation(
                out=tailT2[0:F2, :], in_=pm_t[0:F2, 1, 0:NW], func=gelu,
                bias=b_sb[0:F2, 3:4],
            )

            # ---- step A: uhT_t[j, x] = sum_i UW[t,i,j] headT[i,x]; tag
            # pairs share one PSUM bank so copies are 512 wide ----
            # only vector (DVE) and scalar (ACT) can read PSUM
            def pcopy(k, out, in_):
                if k % 2:
                    nc.scalar.activation(out=out, in_=in_, func=copyf)
                else:
                    nc.vector.tensor_copy(out, in_)

            for jt, (jw, j0, uh) in enumerate(((128, 0, uh1), (I2, 128, uh2))):
                for tp in range(5):
                    t0, tn = 2 * tp, min(2, TAGS - 2 * tp)
                    pa = pslot()
                    for tt in range(tn):
                        for it, (u_sb, h_sb) in enumerate(
                            ((uw1, headT1), (uw2, headT2))
                        ):
                            nc.tensor.matmul(
                                pa[0:jw, tt, 0:NQ],
                                u_sb[:, t0 + tt, j0 : j0 + jw],
                                h_sb,
                                start=(it == 0), stop=(it == 1),
                            )
                    pcopy(
                        jt * 5 + tp,
                        uh[0:jw, t0 : t0 + tn, :], pa[0:jw, 0:tn, 0:NQ]
                    )

            # ---- step B: S[m, (t,x)] = sum_j tailT[j, m] uhT[j, (t,x)];
            # tail slice stationary, uh chunks of 512 moving, 4 groups ----
            # groups: (m-block, x-half): (0, lo), (1, lo), (1, hi), (2, hi)
            groups = ((0, 0), (1, 0), (1, 1), (2, 1))
            # chunks of the 9*128 free dim: (slot, bank, tag0, ntags)
            chunks = ((0, 0, 0, 4), (0, 1, 4, 4), (1, 0, 8, 1))
            sq = (nc.sync, nc.gpsimd, nc.sync, nc.gpsimd)
            for g, (mb, xh) in enumerate(groups):
                m0 = mb * 128
                x0 = xh * 128
                pb = (pslot(), pslot())
                for jt, (jw, tl, uh) in enumerate(
                    ((128, tailT1, uh1), (I2, tailT2, uh2))
                ):
                    for sl, bk, ct, cn in chunks:
                        nc.tensor.matmul(
                            pb[sl][:, bk, 0 : cn * 128],
                            tl[0:jw, m0 : m0 + 128],
                            uh[0:jw, ct : ct + cn, x0 : x0 + 128],
                            start=(jt == 0), stop=(jt == 1),
                        )
                pcopy(g, s_sb[:, g, 0:1024], pb[0].rearrange("p a b -> p (a b)"))
                pcopy(g + 1, s_sb[:, g, 1024:GSZ], pb[1][:, 0, 0:128])
                sq[g].dma_start(out=sout[g], in_=s_sb[:, g, :])

    nc.compile()
    return nc


def _np_io_dt():
    if io_dt_name() == "bf16":
        import ml_dtypes

        return ml_dtypes.bfloat16
    return np.float32


def _get_nc():
    key = "nc-" + io_dt_name() + os.environ.get("BASSK_WARM", "6")
    if key not in _cache:
        _cache[key] = _build_nc()
    return _cache[key]


def _install_ntff_hook():
    """Profiling-only (BASSK_TRACE=1): provide antenv.axon_hooks if the
    image lacks it, wired to the libaxon NTFF capture via ctypes."""
    import sys
    import types

    try:
        from antenv.axon_hooks import get_axon_ntff_profile_hook  # noqa: F401

        return
    except ImportError:
        pass
    from trn_agent_boot.trn_boot import _ntff_profile_via_ctypes

    hook = _ntff_profile_via_ctypes("/opt/axon/libaxon_pjrt.so")
    mod = types.ModuleType("antenv.axon_hooks")
    mod._hook = hook
    mod.get_axon_ntff_profile_hook = lambda: mod._hook
    mod.set_axon_ntff_profile_hook = lambda h: setattr(mod, "_hook", h)
    sys.modules["antenv.axon_hooks"] = mod


def _host_prep(state, Wh, bh, Wt, bt, U, Wcat, Wd):
    """Fold U/Wcat/Wd into UW[9,201,201] and build per-core inputs."""
    iodt = _np_io_dt()
    Whp = Wcat[:, :F]  # [K, 201]
    Wtp = Wcat[:, F:]  # [K, 201]
    U2 = U.astype(np.float64).copy()
    U2[:, F - 1, :] += Wtp  # head ones-row picks up the tail term
    U2[:, :, F - 1] += Whp  # tail ones-col picks up the head term
    UW = np.einsum("kt,kij->tij", Wd.astype(np.float64), U2).astype(np.float32)
    UWt = np.ascontiguousarray(UW.transpose(1, 0, 2))  # [i, t, j]

    whd = np.ascontiguousarray(
        Wh.reshape(6, 128, BSZ).transpose(1, 0, 2)
    ).astype(iodt)
    wtd = np.ascontiguousarray(
        Wt.reshape(6, 128, BSZ).transpose(1, 0, 2)
    ).astype(iodt)
    bias4 = np.ascontiguousarray(
        np.stack(
            [
                bh[0:128],
                bt[0:128],
                np.pad(bh[128:BSZ], (0, 128 - F2)),
                np.pad(bt[128:BSZ], (0, 128 - F2)),
            ],
            axis=1,
        ).astype(np.float32)
    )
    uwflat = UWt.reshape(F, TAGS * F).astype(iodt)  # [i, 1809]
    uwpad = np.zeros((F, 3, 608), iodt)
    uwpad[:, :, 0:603] = uwflat.reshape(F, 3, 603)
    uw1d = np.ascontiguousarray(uwpad[0:128])
    uw2d = np.ascontiguousarray(uwpad[128:F])
    onesd = np.ones((1, NW), iodt)

    in_maps = []
    for b in range(B):
        for qi in range(N // NQ):
            lo = qi * NQ - W
            xw = np.zeros((NW, HID), np.float32)
            s, e = max(lo, 0), min(lo + NW, N)
            xw[s - lo : e - lo] = state[b, s:e]
            xdev = np.ascontiguousarray(
                xw.T.reshape(6, 128, NW).transpose(1, 0, 2)
            ).astype(iodt)
            in_maps.append(
                {
                    "xd": xdev,
                    "whd": whd,
                    "wtd": wtd,
                    "bias4": bias4,
                    "uw1d": uw1d,
                    "uw2d": uw2d,
                    "onesd": onesd,
                }
            )
    return in_maps


def _assemble(outs, bd, lengths):
    """outs: NCORES arrays [4, 128, TAGS*128] -> scores [B, N, R, TAGS]."""
    n_ar = np.arange(N)
    offs = np.arange(R) - W
    j_idx = n_ar[:, None] + offs[None, :]  # [N, R]
    in_range = (j_idx >= 0) & (j_idx < N)
    key_ok = in_range[None] & (j_idx[None] < lengths[:, None, None])
    q_ok = n_ar[None, :] < lengths[:, None]
    pad = ~(key_ok & q_ok[:, :, None])  # [B, N, R]

    xx = np.arange(128)
    idx = (xx[:, None] + np.arange(R)[None, :])[:, :, None]  # [128, R, 1]
    scores = np.empty((B, N, R, TAGS), np.float32)
    for c, S in enumerate(outs):
        b, qi = divmod(c, N // NQ)
        q0 = qi * NQ
        G = S.astype(np.float32).reshape(4, 128, TAGS, 128)
        for half in range(2):
            H = np.concatenate(
                (G[2 * half], G[2 * half + 1]), axis=0
            )  # [256 m, TAGS, 128 xx]
            T = H.transpose(2, 0, 1)  # [xx, m, t]
            band = np.take_along_axis(T, idx, axis=1)  # [128, R, TAGS]
            scores[b, q0 + 128 * half : q0 + 128 * (half + 1)] = band
    scores = np.where(pad[..., None], 0.0, scores) + bd.astype(np.float32)
    return np.where(np.isfinite(scores), scores, 0.0).astype(np.float32)


def kernel(**inputs):
    state = np.asarray(inputs["state"], np.float32)
    lengths = np.asarray(inputs["lengths"]).astype(np.int64)
    Wh = np.ascontiguousarray(np.asarray(inputs["Wh"], np.float32))
    bh = np.asarray(inputs["bh"], np.float32)
    Wt = np.ascontiguousarray(np.asarray(inputs["Wt"], np.float32))
    bt = np.asarray(inputs["bt"], np.float32)
    U = np.asarray(inputs["U"], np.float32)
    Wcat = np.asarray(inputs["Wcat"], np.float32)
    Wd = np.asarray(inputs["Wd"], np.float32)
    bd = np.asarray(inputs["bd"], np.float32)

    in_maps = _host_prep(state, Wh, bh, Wt, bt, U, Wcat, Wd)
    nc = _get_nc()

    if os.environ.get("BASSK_SIM"):
        from concourse.bass_interp import CoreSim

        outs = []
        for im in in_maps:
            sim = CoreSim(nc, trace=False)
            for k, v in im.items():
                sim.tensor(k)[:] = v
            sim.simulate()
            outs.append(sim.tensor("sout").copy())
    else:
        trace = bool(os.environ.get("BASSK_TRACE"))
        if trace:
            _install_ntff_hook()
        from concourse.bass_utils import run_bass_kernel_spmd

        try:
            res = run_bass_kernel_spmd(
                nc, in_maps, core_ids=list(range(NCORES)), trace=trace
            )
        except Exception:
            # transient NRT/device hiccups recover on a fresh attempt
            import time

            time.sleep(2.0)
            res = run_bass_kernel_spmd(
                nc, in_maps, core_ids=list(range(NCORES)), trace=trace
            )
        _cache["last_result"] = res
        outs = [r["sout"] for r in res.results]

    return _assemble(outs, bd, lengths)


# revision 23
# speedup vs baseline: 1.3087x; 1.0591x over previous
"""Trainium2 Bass kernel for nn_CNNNer (sparse band biaffine NER scorer).

Math collapse (everything after the GELU stage is linear):
  head = gelu(state@Wh+bh) ++ [1]          (features i = 0..200, i=200 is the 1)
  tail = gelu(state@Wt+bt) ++ [1]
  band[n,r,k] = head[n]^T U''_k tail[m],  m = n+r-64
      with U''_k = U_k + e_200 Wtp[k,:] + Whp[k,:]^T e_200^T
  scores'[n,r,t] = head[n]^T UW_t tail[m],  UW_t = sum_k Wd[k,t] U''_k
      (precomputed on host, [9,201,201]); scores = scores' + bd.
  Pad masking only ever zeroes whole band entries -> masked scores equal bd
  exactly, so masking moves to the host entirely (device computes garbage in
  masked slots; finite, overwritten on host).

Device work per core (8 cores; core = (batch b, query quarter), 256 queries,
window of NW=384 key positions), all matmuls bf16 with fp32 PSUM accumulate:
  MLP:   headT[f, x] (queries, 256 cols), tailT[f, m] (384 cols), f = 201
         (128 + 73 partition tiles; row 200 memset to 1.0).
  A:     uhT_t[j, x] = sum_i UW[t,i,j] headT[i,x]   (9 tags, 36 matmuls)
  B:     S_t[m, x]   = sum_j tailT[j, m] uhT_t[j, x], computed as 4 groups
         (m-block 0 x-lo, m-block 1 x-lo, m-block 1 x-hi, m-block 2 x-hi),
         tail slice stationary, uh (t,x)-chunks of 512 moving.
Output sout[4, 128, 1152] bf16 = [group][m][(t,x)]; host extracts the
129-wide band diagonals, applies pad mask and + bd.
"""

import os

import numpy as np

B, N, HID = 2, 1024, 768
BSZ = 200
W = 64
TAGS = 9
F = BSZ + 1  # 201 features incl the ones column
NQ = 256  # queries per core
NW = NQ + 2 * W  # 384 window positions per core
R = 2 * W + 1  # 129 band offsets
NCORES = 8
I2 = F - 128  # 73: second feature tile rows (incl ones row at local 72)
F2 = BSZ - 128  # 72: second MLP output tile rows
GSZ = TAGS * 128  # 1152: per-group output elems per partition

_cache: dict = {}


def io_dt_name():
    return os.environ.get("BASSK_IO_DT", "bf16")


def _build_nc():
    import concourse.bass as bass
    import concourse.mybir as mybir
    import concourse.tile as tile
    from concourse import bacc

    dt = mybir.dt
    f32 = dt.float32
    io = {"f32": f32, "f32r": dt.float32r, "bf16": dt.bfloat16}[io_dt_name()]
    nwarm = int(os.environ.get("BASSK_WARM", "6"))

    nc = bacc.Bacc(
        "TRN2", target_bir_lowering=False, debug=False, enable_asserts=False
    )
    # All dram layouts are partition-major with per-partition contiguous runs
    # of 2.3-4.6KB so DMA descriptors are fat and stripe across the 16 DMA
    # engines.
    xd = nc.dram_tensor("xd", [128, 6, NW], io, kind="ExternalInput").ap()
    whd = nc.dram_tensor("whd", [128, 6, BSZ], io, kind="ExternalInput").ap()
    wtd = nc.dram_tensor("wtd", [128, 6, BSZ], io, kind="ExternalInput").ap()
    # bias4 cols: bh[0:128], bt[0:128], bh[128:200]+pad, bt[128:200]+pad
    bias4 = nc.dram_tensor("bias4", [128, 4], f32, kind="ExternalInput").ap()
    # UW packed two i-rows per partition: partition p holds row p (elems
    # 0:1809) and row 128+p (elems 1809:3618, zeros for p >= 73). Stored as
    # 6 padded runs of 603 so each DMA chain is 128 descriptors of 1206B —
    # chains with fewer/fatter descriptors skew onto a single DMA engine.
    uwd = nc.dram_tensor("uwd", [128, 6, 608], io, kind="ExternalInput").ap()
    onesd = nc.dram_tensor("onesd", [1, NW], io, kind="ExternalInput").ap()
    sout = nc.dram_tensor("sout", [4, 128, GSZ], io, kind="ExternalOutput").ap()

    gelu = {
        "gelu": mybir.ActivationFunctionType.Gelu,
        "identity": mybir.ActivationFunctionType.Identity,
    }[os.environ.get("BASSK_ACT", "gelu")]
    copyf = mybir.ActivationFunctionType.Copy

    with tile.TileContext(nc) as tc:
        with (
            tc.tile_pool(name="sb", bufs=1) as sb,
            tc.tile_pool(name="ps", bufs=1, space="PSUM") as ps,
        ):
            x_sb = sb.tile([128, 6, NW], io)
            wh_sb = sb.tile([128, 6, BSZ], io)
            wt_sb = sb.tile([128, 6, BSZ], io)
            b_sb = sb.tile([128, 4], f32)
            uwall = sb.tile([128, 2 * TAGS * F], io)
            headT1 = sb.tile([128, NQ], io)
            headT2 = sb.tile([I2, NQ], io)
            tailT1 = sb.tile([128, NW], io)
            tailT2 = sb.tile([I2, NW], io)
            uh1 = sb.tile([128, TAGS, NQ], io)
            uh2 = sb.tile([I2, TAGS, NQ], io)
            s_sb = sb.tile([128, 4, GSZ], io)
            warm = sb.tile([128, 512], io)

            # ---- loads: head inputs on sync, tail weights on scalar, uw
            # chains on gpsimd — three queues streaming concurrently, each
            # chain 128 fat descriptors, ordered by first use ----
            nc.gpsimd.dma_start(out=headT2[F2:I2, :], in_=onesd[:, 0:NQ])
            nc.gpsimd.dma_start(out=tailT2[F2:I2, :], in_=onesd)
            nc.gpsimd.dma_start(out=b_sb, in_=bias4)
            for k in range(6):
                nc.gpsimd.dma_start(
                    out=uwall[:, 603 * k : 603 * (k + 1)],
                    in_=uwd[:, k, 0:603],
                )
            nc.sync.dma_start(out=x_sb[:, 0:3, :], in_=xd[:, 0:3, :])
            nc.sync.dma_start(out=wh_sb[:, 0:3, :], in_=whd[:, 0:3, :])
            nc.sync.dma_start(out=wh_sb[:, 3:6, :], in_=whd[:, 3:6, :])
            nc.sync.dma_start(out=x_sb[:, 3:6, :], in_=xd[:, 3:6, :])
            nc.scalar.dma_start(out=wt_sb[:, 0:3, :], in_=wtd[:, 0:3, :])
            nc.scalar.dma_start(out=wt_sb[:, 3:6, :], in_=wtd[:, 3:6, :])

            nc.vector.memset(warm, 0.5)

            # All PSUM comes from one tag of [128, 1024]-f32 (2-bank) slots,
            # 4 bufs = all 8 banks. Each accumulation group starts at a bank
            # boundary (a group's start-flag zeroes its whole 2KB bank).
            def pslot():
                return ps.tile([128, 2, 512], f32, tag="u", bufs=4, name="pu")

            # ---- PE clock warmup: dependency-free matmuls run while the
            # input DMAs stream, ramping the tensor engine out of its low
            # p-state before the real work arrives ----
            if nwarm:
                pw = pslot()
                for k in range(nwarm):
                    nc.tensor.matmul(
                        pw[:, 0, :], warm[:, 0:128], warm,
                        start=(k == 0), stop=(k == nwarm - 1),
                    )

            # ---- MLPs: o = gelu(W^T x + b), transposed layout; head (12
            # matmuls) first so step A can start while the tail runs ----
            pm_h = pslot()
            pm_t = pslot()
            for ht in range(6):
                fl = (ht == 0, ht == 5)
                nc.tensor.matmul(
                    pm_h[:, 0, 0:NQ], wh_sb[:, ht, 0:128],
                    x_sb[:, ht, W : W + NQ], start=fl[0], stop=fl[1],
                )
                nc.tensor.matmul(
                    pm_h[0:F2, 1, 0:NQ], wh_sb[:, ht, 128:BSZ],
                    x_sb[:, ht, W : W + NQ], start=fl[0], stop=fl[1],
                )
            nc.scalar.activation(
                out=headT1, in_=pm_h[:, 0, 0:NQ], func=gelu, bias=b_sb[:, 0:1]
            )
            nc.scalar.activation(
                out=headT2[0:F2, :], in_=pm_h[0:F2, 1, 0:NQ], func=gelu,
                bias=b_sb[0:F2, 2:3],
            )
            for ht in range(6):
                fl = (ht == 0, ht == 5)
                nc.tensor.matmul(
                    pm_t[:, 0, 0:NW], wt_sb[:, ht, 0:128], x_sb[:, ht, :],
                    start=fl[0], stop=fl[1],
                )
                nc.tensor.matmul(
                    pm_t[0:F2, 1, 0:NW], wt_sb[:, ht, 128:BSZ], x_sb[:, ht, :],
                    start=fl[0], stop=fl[1],
                )
            nc.scalar.activation(
                out=tailT1, in_=pm_t[:, 0, 0:NW], func=gelu, bias=b_sb[:, 1:2]
            )
            nc.scalar.activation(
                out=tailT2[0:F2, :], in_=pm_t[0:F2, 1, 0:NW], func=gelu,
                bias=b_sb[0:F2, 3:4],
            )

            # ---- step A: uhT_t[j, x] = sum_i UW[t,i,j] headT[i,x]; tag
            # pairs share one PSUM bank so copies are 512 wide ----
            # only vector (DVE) and scalar (ACT) can read PSUM
            def pcopy(k, out, in_):
                if k % 2:
                    nc.scalar.activation(out=out, in_=in_, func=copyf)
                else:
                    nc.vector.tensor_copy(out, in_)

            TF = TAGS * F
            for jt, (jw, j0, uh) in enumerate(((128, 0, uh1), (I2, 128, uh2))):
                for tp in range(5):
                    t0, tn = 2 * tp, min(2, TAGS - 2 * tp)
                    pa = pslot()
                    for tt in range(tn):
                        off = (t0 + tt) * F + j0
                        for it, (ub, ip, h_sb) in enumerate(
                            ((uwall, 128, headT1), (uwall, I2, headT2))
                        ):
                            nc.tensor.matmul(
                                pa[0:jw, tt, 0:NQ],
                                ub[0:ip, it * TF + off : it * TF + off + jw],
                                h_sb,
                                start=(it == 0), stop=(it == 1),
                            )
                    pcopy(
                        jt * 5 + tp,
                        uh[0:jw, t0 : t0 + tn, :], pa[0:jw, 0:tn, 0:NQ]
                    )

            # ---- step B: S[m, (t,x)] = sum_j tailT[j, m] uhT[j, (t,x)];
            # tail slice stationary, uh chunks of 512 moving, 4 groups ----
            # groups: (m-block, x-half): (0, lo), (1, lo), (1, hi), (2, hi)
            groups = ((0, 0), (1, 0), (1, 1), (2, 1))
            # chunks of the 9*128 free dim: (slot, bank, tag0, ntags)
            chunks = ((0, 0, 0, 4), (0, 1, 4, 4), (1, 0, 8, 1))
            sq = (nc.sync, nc.gpsimd, nc.sync, nc.gpsimd)
            for g, (mb, xh) in enumerate(groups):
                m0 = mb * 128
                x0 = xh * 128
                pb = (pslot(), pslot())
                for jt, (jw, tl, uh) in enumerate(
                    ((128, tailT1, uh1), (I2, tailT2, uh2))
                ):
                    for sl, bk, ct, cn in chunks:
                        nc.tensor.matmul(
                            pb[sl][:, bk, 0 : cn * 128],
                            tl[0:jw, m0 : m0 + 128],
                            uh[0:jw, ct : ct + cn, x0 : x0 + 128],
                            start=(jt == 0), stop=(jt == 1),
                        )
                pcopy(g, s_sb[:, g, 0:1024], pb[0].rearrange("p a b -> p (a b)"))
                pcopy(g + 1, s_sb[:, g, 1024:GSZ], pb[1][:, 0, 0:128])
                sq[g].dma_start(out=sout[g], in_=s_sb[:, g, :])

    nc.compile()
    return nc


def _np_io_dt():
    if io_dt_name() == "bf16":
        import ml_dtypes

        return ml_dtypes.bfloat16
    return np.float32


def _get_nc():
    key = "nc-" + io_dt_name() + os.environ.get("BASSK_WARM", "6")
    if key not in _cache:
        _cache[key] = _build_nc()
    return _cache[key]


def _install_ntff_hook():
    """Profiling-only (BASSK_TRACE=1): provide antenv.axon_hooks if the
    image lacks it, wired to the libaxon NTFF capture via ctypes."""
    import sys
    import types

    try:
        from antenv.axon_hooks import get_axon_ntff_profile_hook  # noqa: F401

        return
    except ImportError:
        pass
    from trn_agent_boot.trn_boot import _ntff_profile_via_ctypes

    hook = _ntff_profile_via_ctypes("/opt/axon/libaxon_pjrt.so")
    mod = types.ModuleType("antenv.axon_hooks")
    mod._hook = hook
    mod.get_axon_ntff_profile_hook = lambda: mod._hook
    mod.set_axon_ntff_profile_hook = lambda h: setattr(mod, "_hook", h)
    sys.modules["antenv.axon_hooks"] = mod


def _host_prep(state, Wh, bh, Wt, bt, U, Wcat, Wd):
    """Fold U/Wcat/Wd into UW[9,201,201] and build per-core inputs."""
    iodt = _np_io_dt()
    Whp = Wcat[:, :F]  # [K, 201]
    Wtp = Wcat[:, F:]  # [K, 201]
    U2 = U.astype(np.float64).copy()
    U2[:, F - 1, :] += Wtp  # head ones-row picks up the tail term
    U2[:, :, F - 1] += Whp  # tail ones-col picks up the head term
    UW = np.einsum("kt,kij->tij", Wd.astype(np.float64), U2).astype(np.float32)
    UWt = np.ascontiguousarray(UW.transpose(1, 0, 2))  # [i, t, j]

    whd = np.ascontiguousarray(
        Wh.reshape(6, 128, BSZ).transpose(1, 0, 2)
    ).astype(iodt)
    wtd = np.ascontiguousarray(
        Wt.reshape(6, 128, BSZ).transpose(1, 0, 2)
    ).astype(iodt)
    bias4 = np.ascontiguousarray(
        np.stack(
            [
                bh[0:128],
                bt[0:128],
                np.pad(bh[128:BSZ], (0, 128 - F2)),
                np.pad(bt[128:BSZ], (0, 128 - F2)),
            ],
            axis=1,
        ).astype(np.float32)
    )
    uwflat = UWt.reshape(F, TAGS * F)  # [i, 1809]
    uwcat = np.zeros((128, 2 * TAGS * F), np.float32)
    uwcat[:, 0 : TAGS * F] = uwflat[0:128]
    uwcat[0:I2, TAGS * F :] = uwflat[128:F]
    uwd = np.zeros((128, 6, 608), iodt)
    uwd[:, :, 0:603] = uwcat.reshape(128, 6, 603).astype(iodt)
    onesd = np.ones((1, NW), iodt)

    in_maps = []
    for b in range(B):
        for qi in range(N // NQ):
            lo = qi * NQ - W
            xw = np.zeros((NW, HID), np.float32)
            s, e = max(lo, 0), min(lo + NW, N)
            xw[s - lo : e - lo] = state[b, s:e]
            xdev = np.ascontiguousarray(
                xw.T.reshape(6, 128, NW).transpose(1, 0, 2)
            ).astype(iodt)
            in_maps.append(
                {
                    "xd": xdev,
                    "whd": whd,
                    "wtd": wtd,
                    "bias4": bias4,
                    "uwd": uwd,
                    "onesd": onesd,
                }
            )
    return in_maps


def _assemble(outs, bd, lengths):
    """outs: NCORES arrays [4, 128, TAGS*128] -> scores [B, N, R, TAGS]."""
    n_ar = np.arange(N)
    offs = np.arange(R) - W
    j_idx = n_ar[:, None] + offs[None, :]  # [N, R]
    in_range = (j_idx >= 0) & (j_idx < N)
    key_ok = in_range[None] & (j_idx[None] < lengths[:, None, None])
    q_ok = n_ar[None, :] < lengths[:, None]
    pad = ~(key_ok & q_ok[:, :, None])  # [B, N, R]

    xx = np.arange(128)
    idx = (xx[:, None] + np.arange(R)[None, :])[:, :, None]  # [128, R, 1]
    scores = np.empty((B, N, R, TAGS), np.float32)
    for c, S in enumerate(outs):
        b, qi = divmod(c, N // NQ)
        q0 = qi * NQ
        G = S.astype(np.float32).reshape(4, 128, TAGS, 128)
        for half in range(2):
            H = np.concatenate(
                (G[2 * half], G[2 * half + 1]), axis=0
            )  # [256 m, TAGS, 128 xx]
            T = H.transpose(2, 0, 1)  # [xx, m, t]
            band = np.take_along_axis(T, idx, axis=1)  # [128, R, TAGS]
            scores[b, q0 + 128 * half : q0 + 128 * (half + 1)] = band
    scores = np.where(pad[..., None], 0.0, scores) + bd.astype(np.float32)
    return np.where(np.isfinite(scores), scores, 0.0).astype(np.float32)


def kernel(**inputs):
    state = np.asarray(inputs["state"], np.float32)
    lengths = np.asarray(inputs["lengths"]).astype(np.int64)
    Wh = np.ascontiguousarray(np.asarray(inputs["Wh"], np.float32))
    bh = np.asarray(inputs["bh"], np.float32)
    Wt = np.ascontiguousarray(np.asarray(inputs["Wt"], np.float32))
    bt = np.asarray(inputs["bt"], np.float32)
    U = np.asarray(inputs["U"], np.float32)
    Wcat = np.asarray(inputs["Wcat"], np.float32)
    Wd = np.asarray(inputs["Wd"], np.float32)
    bd = np.asarray(inputs["bd"], np.float32)

    in_maps = _host_prep(state, Wh, bh, Wt, bt, U, Wcat, Wd)
    nc = _get_nc()

    if os.environ.get("BASSK_SIM"):
        from concourse.bass_interp import CoreSim

        outs = []
        for im in in_maps:
            sim = CoreSim(nc, trace=False)
            for k, v in im.items():
                sim.tensor(k)[:] = v
            sim.simulate()
            outs.append(sim.tensor("sout").copy())
    else:
        trace = bool(os.environ.get("BASSK_TRACE"))
        if trace:
            _install_ntff_hook()
        from concourse.bass_utils import run_bass_kernel_spmd

        try:
            res = run_bass_kernel_spmd(
                nc, in_maps, core_ids=list(range(NCORES)), trace=trace
            )
        except Exception:
            # transient NRT/device hiccups recover on a fresh attempt
            import time

            time.sleep(2.0)
            res = run_bass_kernel_spmd(
                nc, in_maps, core_ids=list(range(NCORES)), trace=trace
            )
        _cache["last_result"] = res
        outs = [r["sout"] for r in res.results]

    return _assemble(outs, bd, lengths)
